# revision 1
# baseline (speedup 1.0000x reference)
"""GCL (GNN message-passing) Trainium2 Bass kernel on 8 NeuronCores.

Sharding: edges sorted by destination on host and sharded by destination-node
range (1250 nodes/core) -> each core owns the full segment-sum for its nodes,
no collectives. Node features and weights replicated.

Per core, the edge-MLP first-layer split: e1 = A[row] + B[col] where
A = h@we1_top + be1 (per-window SBUF bf16 table, injected via one-hot
matmul) and B = h@we1_bot (bf16 table RESIDENT IN SBUF, 2.56MB; per-edge
rows delivered by SBUF-source dma_gather in transpose mode, which lands
them directly in [D, e] layout -> single identity-matmul inject).

Per core, per 128-destination-node window, edges are processed in 512-edge
macro tiles:
  e1T[D,e] = A @ S_T + I @ BT_gathered               (PSUM accumulate)
  where S_T[n,e] = one-hot(row_local[e]==n) via DVE is_equal (bf16, 4x)
  e2[e,D] = silu(e1) @ we2 + be2                     (be2 via K=1 ones x be2)
  aggT[D,n] += e2^T-scatter via lhsT=e2s, rhs=S      (PSUM accumulate)
Node MLP + residual per 128-node tile, fp32.
"""
import sys
sys.path.insert(0, '/opt/trn_rl_repo')
import numpy as np
import ml_dtypes

N_NODES = 10000
N_EDGES = 640000
D = 128
NORM = 100.0
NCORES = 8
NPC = N_NODES // NCORES          # 1250 destination nodes per core
NWIN = 10                        # 128-node windows per core
CALL = 4096                      # edges per dma_gather call (= 8 macros)
MPC = 8                          # macros per gather call
CW16 = CALL // 16                # colidx columns per call
MACRO = 512
PAD_ROWLOCAL = 200.0
NB = 80                          # B table groups: 80*128 = 10240 rows

BF16 = ml_dtypes.bfloat16
_prog_cache = {}


def _wrap_idx16(idx):
    """[n] int -> [128, n/16] int16 wrapped (pos i -> partition i%16, col
    i//16) and replicated into all eight 16-partition groups."""
    n = idx.shape[0]
    block = idx.astype(np.int16).reshape(n // 16, 16).T
    return np.tile(block, (8, 1))


QPAT = [0, 1, 0, 2, 0, 1, 0, 3]   # legacy; queues now cc % 4


def _call_plan(NM):
    """Gather call sizes: small first call (2 macros) for fast pipeline
    start, then 4096-edge (8-macro) calls."""
    sizes = [1024]
    covered = 2
    while covered < NM:
        sizes.append(4096)
        covered += 8
    off16 = np.cumsum([0] + [s // 16 for s in sizes]).tolist()
    moff = np.cumsum([0] + [s // 512 for s in sizes]).tolist()
    return sizes, off16, moff


def _build_program(cw_per_window, no_gather=False, no_compute=False,
                   bufs_g=5, bufs_w=4):
    import concourse.bacc as bacc
    import concourse.mybir as mybir
    from concourse import tile

    dt = mybir.dt
    AF = mybir.ActivationFunctionType
    ALU = mybir.AluOpType

    nm_w = list(cw_per_window)       # macros (512 edges) per window
    NM = sum(nm_w)
    SIZES, OFF16, MOFF = _call_plan(NM)
    NCALLS = len(SIZES)
    TOT16 = OFF16[-1]

    nc = bacc.Bacc("TRN2", target_bir_lowering=False, debug=False,
                   num_devices=NCORES, num_swdge_queues=4)

    f32, bf16, i16, u8 = dt.float32, dt.bfloat16, dt.int16, dt.uint8
    din = lambda n, s, d=f32: nc.dram_tensor(n, s, d, kind="ExternalInput")
    hT_bf = din("hT_bf", [128, NB * 128], bf16)
    hTs_bf_d = din("hTs_bf", [128, NWIN * 128], bf16)
    h_slice = din("h_slice", [NWIN, 128, 128])
    we1_top_bf = din("we1_top_bf", [128, 128], bf16)
    we1_bot_bf = din("we1_bot_bf", [128, 128], bf16)
    be1_bf_d = din("be1_bf", [1, 128], bf16)
    we2b_d = din("we2_bf", [128, 128], bf16)
    be2rep4_bf = din("be2rep4_bf", [1, 512], bf16)
    wn1_lo_bf = din("wn1_lo_bf", [128, 128], bf16)
    wn1_hi_bf = din("wn1_hi_bf", [128, 128], bf16)
    bn1_col = din("bn1_col", [128, 1])
    wn2_bf_d = din("wn2_bf", [128, 128], bf16)
    bn2_bf_d = din("bn2_bf", [1, 128], bf16)
    ones_bf_d = din("ones_bf", [1, 128], bf16)
    iota_col_bf_d = din("iota_col_bf", [128, 512], bf16)
    iota_part_d = din("iota_part", [128, 1])
    ident_bf_d = din("ident_bf", [128, 128], bf16)
    ident_f_d = din("ident_f", [128, 128])
    norm_col_d = din("norm_col", [128, 1])
    colidx_d = din("colidx", [128, TOT16], i16)
    rowloc_c_d = din("rowloc_c", [128, 4 * NM])
    rowloc_r_u8_d = din("rowloc_r_u8", [NM, 512], u8)
    out_d = nc.dram_tensor("out", [NWIN, 128, 128], f32, kind="ExternalOutput")

    with tile.TileContext(nc) as tc:
        with (
            tc.tile_pool(name="persist", bufs=1) as pp,
            tc.tile_pool(name="work", bufs=bufs_w) as wp,
            tc.tile_pool(name="gout", bufs=bufs_g) as gp,
            tc.tile_pool(name="ps1", bufs=3, space="PSUM") as psp1,
            tc.tile_pool(name="ps2", bufs=2, space="PSUM") as psp2,
            tc.tile_pool(name="psa", bufs=1, space="PSUM") as pspa,
        ):
            def load(t_dram, shape, dtype=f32):
                t = pp.tile(shape, dtype, tag=t_dram.name)
                nc.sync.dma_start(t[:], t_dram.ap())
                return t

            hTb_t = pp.tile([128, NB * 128], bf16, tag="hT_bf")
            for hc in range(10):
                nc.sync.dma_start(hTb_t[:, hc * 1024:(hc + 1) * 1024],
                                  hT_bf.ap()[:, hc * 1024:(hc + 1) * 1024])
            hTs_t = load(hTs_bf_d, [128, NWIN * 128], bf16)
            colidx_t = load(colidx_d, [128, TOT16], i16)
            rowloc_c = load(rowloc_c_d, [128, 4 * NM])
            w1t = load(we1_top_bf, [128, 128], bf16)
            w1b = load(we1_bot_bf, [128, 128], bf16)
            be1r = load(be1_bf_d, [1, 128], bf16)
            w2b = load(we2b_d, [128, 128], bf16)
            be2r = load(be2rep4_bf, [1, 512], bf16)
            wn1l = load(wn1_lo_bf, [128, 128], bf16)
            wn1h = load(wn1_hi_bf, [128, 128], bf16)
            bn1c = load(bn1_col, [128, 1])
            wn2t = load(wn2_bf_d, [128, 128], bf16)
            bn2r = load(bn2_bf_d, [1, 128], bf16)
            onesb = load(ones_bf_d, [1, 128], bf16)
            iota_col = load(iota_col_bf_d, [128, 512], bf16)
            iota_part = load(iota_part_d, [128, 1])
            ident_bf = load(ident_bf_d, [128, 128], bf16)
            ident_f = load(ident_f_d, [128, 128])
            norm_col = load(norm_col_d, [128, 1])
            hsl_t = pp.tile([128, NWIN, 128], f32, tag="h_slice")
            nc.sync.dma_start(hsl_t[:], h_slice.ap().rearrange("w p d -> p w d"))



            # ---- B table: h @ we1_bot -> bf16, SBUF resident ----
            B_sb = pp.tile([128, NB, 128], bf16, tag="B_sb")
            for q in range(NB // 4):
                bp = psp1.tile([128, 512], f32, tag="e1")
                for j in range(4):
                    t = q * 4 + j
                    nc.tensor.matmul(bp[:, j * 128:(j + 1) * 128],
                                     hTb_t[:, t * 128:(t + 1) * 128], w1b[:],
                                     start=True, stop=True,
                                     skip_group_check=True)
                bdst = B_sb[:, q * 4:(q + 1) * 4, :].rearrange(
                    "p a b -> p (a b)")
                if q % 2 == 0:
                    nc.scalar.activation(bdst, bp[:], AF.Copy)
                else:
                    nc.vector.tensor_copy(bdst, bp[:])

            # ---- A table: h @ we1_top + be1, single bf16, SBUF resident ----
            a_bf = pp.tile([128, NWIN, 128], bf16, tag="a_bf")
            for w in range(NWIN):
                ap_ = psp2.tile([128, 128], f32, tag="e2")
                nc.tensor.matmul(ap_[:], onesb[:], be1r[:], start=True, stop=False)
                nc.tensor.matmul(ap_[:], hTs_t[:, w * 128:(w + 1) * 128], w1t[:],
                                 start=False, stop=True)
                nc.scalar.activation(a_bf[:, w, :], ap_[:], AF.Copy)

            # ---- edge phase ----
            agg_sb = None
            if not no_compute:
                agg_sb = pp.tile([128, NWIN, 128], bf16, tag="aggsb")

            # flat macro list: (window, mw-in-window, nmw)
            macros = [(w, mw, nm_w[w]) for w in range(NWIN) for mw in range(nm_w[w])]
            NMtot = len(macros)
            NCH = (NMtot + 7) // 8            # rb chunks of 8 macros
            gts = {}
            rbs = {}
            agg_tiles = {}
            stash = {}
            PREF = 4

            def issue_gather(cc):
                if cc >= NCALLS:
                    return
                sz = SIZES[cc]
                gt = gp.tile([128, 1, sz], bf16,
                             tag="g0" if sz == 1024 else "g")
                if not no_gather:
                    q = cc % 4
                    nc.gpsimd.dma_gather(
                        gt[:], B_sb[:].rearrange("p a b -> p (a b)"),
                        colidx_t[:, OFF16[cc]:OFF16[cc + 1]],
                        num_idxs=sz, num_idxs_reg=sz, elem_size=128,
                        transpose=True, single_packet=False,
                        queue_num=q,
                        sbuf_tokens_per_rank=128,
                        sbuf_free_dim_per_rank=256,
                        sbuf_free_dim_pad_per_rank=0,
                        sbuf_byte_offset=0,
                    )
                else:
                    nc.vector.tensor_copy(gt[:, 0, 0:8], ident_bf[:, 0:8])
                gts[cc] = gt

            def issue_rb(j):
                if j >= NCH or no_compute:
                    return
                n8 = min(8, NMtot - 8 * j)
                rb = wp.tile([128, 8, 512], u8, tag="rb")
                src = rowloc_r_u8_d.ap()[8 * j:8 * j + n8, :].rearrange(
                    "(o a) b -> o (a b)", o=1).broadcast_to((128, n8 * 512))
                nc.sync.dma_start(rb[:, 0:n8, :], src)
                rbs[j] = rb

            for p in range(PREF):
                issue_gather(p)
            for p in range(2):
                issue_rb(p)

            def front(i):
                w, mw, nmw = macros[i]
                ci = 0 if i < 2 else 1 + (i - 2) // 8
                sl = i if i < 2 else (i - 2) % 8
                if sl == 0:
                    issue_gather(ci + PREF)
                gt = gts[ci]
                if no_compute:
                    if sl == 0:
                        sink = wp.tile([128, 8], bf16, tag="sink")
                        nc.vector.tensor_copy(sink[:], gt[:, 0, 0:8])
                    return
                if i % 8 == 0:
                    issue_rb(i // 8 + 2)
                rb = rbs[i // 8]
                st = wp.tile([128, 512], bf16, tag="st")
                nc.vector.tensor_scalar(
                    st[:], rb[:, i % 8, :], iota_part[:, 0:1], None, ALU.is_equal)
                e1p = psp1.tile([128, 512], f32, tag="e1")
                nc.tensor.matmul(e1p[:], ident_bf[:],
                                 gt[:, 0, sl * 512:sl * 512 + 512],
                                 start=True, stop=False, skip_group_check=True)
                nc.tensor.matmul(e1p[:], a_bf[:, w, :], st[:],
                                 start=False, stop=True, skip_group_check=True)
                e1s = wp.tile([128, 512], bf16, tag="e1s")
                nc.scalar.activation(e1s[:], e1p[:], AF.Silu)
                stash[i] = e1s

            pair = {}

            def back1(i):
                if no_compute:
                    return
                e1s = stash.pop(i)
                eoff = 0
                s4 = wp.tile([128, 512], bf16, tag="s4")
                for t in range(4):
                    nc.vector.tensor_scalar(
                        s4[:, t * 128:(t + 1) * 128],
                        iota_col[:, t * 128:(t + 1) * 128],
                        rowloc_c[:, 4 * i + t:4 * i + t + 1],
                        None, ALU.is_equal)
                if i % 2 == 0:
                    e2p = psp2.tile([128, 1024], f32, tag="e2")
                    pair[i // 2] = e2p
                e2p = pair[i // 2]
                off = (i % 2) * 512
                nc.tensor.matmul(e2p[:, off:off + 512], onesb[:], be2r[:],
                                 start=True, stop=False, skip_group_check=True)
                for t in range(4):
                    nc.tensor.matmul(
                        e2p[:, off + t * 128:off + (t + 1) * 128],
                        e1s[:, eoff + t * 128:eoff + (t + 1) * 128], w2b[:],
                        start=False, stop=True, skip_group_check=True)
                stash[("s4", i)] = s4
                if i % 2 == 1 or i == NMtot - 1:
                    e2s = wp.tile([128, 1024], bf16, tag="e2s")
                    if i % 2 == 1:
                        nc.scalar.activation(e2s[:], e2p[:], AF.Silu)
                    else:
                        nc.scalar.activation(e2s[:, 0:512], e2p[:, 0:512],
                                             AF.Silu)
                    stash[("e2s", i // 2)] = e2s

            def back2(i):
                if no_compute:
                    return
                w, mw, nmw = macros[i]
                e2s = stash[("e2s", i // 2)]
                s4 = stash.pop(("s4", i))
                if i % 2 == 1 or i == NMtot - 1:
                    stash.pop(("e2s", i // 2))
                    pair.pop(i // 2)
                off = (i % 2) * 512
                if mw == 0:
                    agg_new = pspa.tile([128, 128], f32, tag="agg")
                    agg_tiles[w] = agg_new
                agg_ps = agg_tiles[w]
                for t in range(4):
                    nc.tensor.matmul(
                        agg_ps[:],
                        e2s[:, off + t * 128:off + (t + 1) * 128],
                        s4[:, t * 128:(t + 1) * 128],
                        start=(mw == 0 and t == 0),
                        stop=(mw == nmw - 1 and t == 3),
                        skip_group_check=True)
                if mw == nmw - 1:
                    nc.vector.tensor_scalar(agg_sb[:, w, :], agg_ps[:],
                                            norm_col[:, 0:1], None, ALU.mult)
                    node_phase(w)

            def node_phase(w):
                hp = psp1.tile([128, 128], f32, tag="e1")
                nc.tensor.matmul(hp[:], wn1l[:], hTs_t[:, w * 128:(w + 1) * 128],
                                 start=True, stop=False)
                nc.tensor.matmul(hp[:], wn1h[:], agg_sb[:, w, :],
                                 start=False, stop=True)
                hs = wp.tile([128, 128], bf16, tag="hs")
                nc.scalar.activation(hs[:], hp[:], AF.Silu, bias=bn1c[:, 0:1])
                op = psp2.tile([128, 128], f32, tag="e2")
                nc.tensor.matmul(op[:], onesb[:], bn2r[:], start=True, stop=False)
                nc.tensor.matmul(op[:], hs[:], wn2t[:], start=False, stop=True)
                ot = wp.tile([128, 128], f32, tag="ot")
                nc.vector.tensor_tensor(ot[:], op[:], hsl_t[:, w, :], ALU.add)
                nc.sync.dma_start(out_d.ap()[w], ot[:])

            for i in range(NMtot + 3):
                if i < NMtot:
                    front(i)
                if 1 <= i <= NMtot:
                    back1(i - 1)
                if i >= 3:
                    back2(i - 3)

            # ---- node phase (interleaved per-window via back2) ----
            if no_compute:
                for w in range(NWIN):
                    nc.sync.dma_start(out_d.ap()[w], hsl_t[:, w, :])

    nc.compile()
    return nc


def _prep_inputs(h, edge_index, we1, be1, we2, be2, wn1, bn1, wn2, bn2):
    """Host-side shard/sort/pad. Returns (cw_per_window, per-core in_maps)."""
    h = np.asarray(h, np.float32)
    row = np.asarray(edge_index[0], np.int64).astype(np.int32)
    col = np.asarray(edge_index[1], np.int64).astype(np.int32)

    # per (core, window) edge lists
    core = row // NPC
    rl_g = row - core * NPC
    win = rl_g // 128
    rl = (rl_g % 128).astype(np.float32)

    counts = np.zeros((NCORES, NWIN), np.int64)
    per = [[None] * NWIN for _ in range(NCORES)]
    for cid in range(NCORES):
        msk = core == cid
        w_c, rl_c, col_c = win[msk], rl[msk], col[msk]
        for w in range(NWIN):
            wm = w_c == w
            per[cid][w] = (col_c[wm], rl_c[wm])
            counts[cid, w] = wm.sum()
    cw_per_window = tuple(int(-(-counts[:, w].max() // MACRO)) for w in range(NWIN))

    nm_w = list(cw_per_window)
    NM = sum(nm_w)
    SIZES, OFF16, MOFF = _call_plan(NM)
    NCALLS = len(SIZES)

    hT_pad = np.zeros((128, NB * 128), np.float32)
    hT_pad[:, :N_NODES] = h.T
    iota_col = np.tile(np.arange(128, dtype=np.float32), 4)[None, :].repeat(128, 0)
    iota_part = np.arange(128, dtype=np.float32)[:, None].copy()
    shared = {
        "hT_bf": hT_pad.astype(BF16),
        "we1_top_bf": np.asarray(we1[:128], np.float32).astype(BF16),
        "we1_bot_bf": np.asarray(we1[128:], np.float32).astype(BF16),
        "be1_bf": np.asarray(be1, np.float32)[None, :].astype(BF16),
        "be2rep4_bf": np.tile(np.asarray(be2, np.float32), 4)[None, :].astype(BF16),
        "wn1_lo_bf": np.asarray(wn1[:128], np.float32).astype(BF16),
        "wn1_hi_bf": np.asarray(wn1[128:], np.float32).astype(BF16),
        "bn1_col": np.asarray(bn1, np.float32)[:, None].copy(),
        "wn2_bf": np.asarray(wn2, np.float32).astype(BF16),
        "bn2_bf": np.asarray(bn2, np.float32)[None, :].astype(BF16),
        "ones_bf": np.ones((1, 128), np.float32).astype(BF16),
        "iota_col_bf": iota_col.astype(BF16),
        "iota_part": iota_part,
        "ident_bf": np.eye(128, dtype=np.float32).astype(BF16),
        "ident_f": np.eye(128, dtype=np.float32),
        "norm_col": np.full((128, 1), 1.0 / NORM, np.float32),
        "we2_bf": np.asarray(we2, np.float32).astype(BF16),
    }

    in_maps = []
    for cid in range(NCORES):
        # flat 512-slot macro stream across all windows
        col_all = np.zeros(OFF16[-1] * 16, np.int32)
        rl_all = np.full(NM * MACRO, PAD_ROWLOCAL, np.float32)
        pos = 0
        for w in range(NWIN):
            ccol, crl = per[cid][w]
            col_all[pos:pos + len(ccol)] = ccol
            rl_all[pos:pos + len(crl)] = crl
            pos += nm_w[w] * MACRO
        colidx = np.zeros((128, OFF16[-1]), np.int16)
        for cc in range(NCALLS):
            colidx[:, OFF16[cc]:OFF16[cc + 1]] = _wrap_idx16(
                col_all[OFF16[cc] * 16:OFF16[cc + 1] * 16])
        rowloc_c = np.zeros((128, 4 * NM), np.float32)
        rowloc_r = rl_all.reshape(NM, MACRO)
        for mi in range(NM):
            rowloc_c[:, 4 * mi:4 * mi + 4] = rowloc_r[mi].reshape(4, 128).T
        base = cid * NPC
        hTs_bf = hT_pad[:, base:base + NWIN * 128].astype(BF16)
        h_slice = np.zeros((NWIN, 128, 128), np.float32)
        hi = min(N_NODES, base + NWIN * 128)
        h_slice.reshape(NWIN * 128, 128)[:hi - base] = h[base:hi]
        in_maps.append({**shared, "hTs_bf": hTs_bf, "h_slice": h_slice,
                        "colidx": colidx, "rowloc_c": rowloc_c,
                        "rowloc_r_u8": rowloc_r.astype(np.uint8)})
    return cw_per_window, in_maps


def kernel(**inputs):
    from concourse.bass_utils import run_bass_kernel_spmd

    cw, in_maps = _prep_inputs(**inputs)
    if cw not in _prog_cache:
        _prog_cache[cw] = _build_program(cw)
    nc = _prog_cache[cw]
    res = run_bass_kernel_spmd(nc, in_maps, list(range(NCORES)))
    outs = []
    for cid in range(NCORES):
        o = res.results[cid]["out"].reshape(NWIN * 128, 128)
        outs.append(o[:NPC])
    return np.concatenate(outs, axis=0)[:N_NODES].astype(np.float32)

